# revision 15
# baseline (speedup 1.0000x reference)
"""Trainium2 Bass kernel for nn_FCFClient (scatter_memory).

reference(X, global_Y, likes, movie_ids) -> (avg_loss, y_grad):
    s     = X @ global_Y                    # (I,) per-item scores
    err_j = s[id_j] - likes_j               # per interaction
    loss  = mean(err^2)
    y_grad[:, m] = (2*err_win(m)*X.T + 2*lam*Y[:, m]) / N   for touched m (last
                   occurrence wins, CPU-jax scatter semantics), else 0.

All per-interaction work is compressed host-side (pure index manipulation /
bincount over ids+likes) into dense per-column arrays:
    cnt[m], sl[m]=sum likes, sl2[m]=sum likes^2, win_like[m], touched[m]
so that
    loss = (sum_m cnt*s^2 - 2*sl*s + sl2) / N
    y_grad[:, m] = c[m]*X.T + t2[m]*Y[:, m]
        c[m]  = s[m]*tch2[m] - wl2[m],  tch2 = touched*2/N, wl2 = win_like*tch2
        t2[m] = touched[m]*2*lam/N

Device (per core, column shard of 16384 items): stream Y, PE computes s
(column-tiled) and the rank-1 c (x) X term, DVE applies y*t2 + rank1, fused
reductions produce the partial loss. Memory roofline: read Y + write y_grad.
"""

import numpy as np

import concourse.bass as bass
import concourse.tile as tile
from concourse import bacc, mybir, bass_utils
from concourse.masks import make_identity

K = 256
I = 131072
N = 262144
NCORES = 8
M = I // NCORES          # 16384 columns per core
LAMBDA_REG = 0.01
F = 2048                 # columns per chunk
NCH = M // F             # 8 chunks
JT = F // 128            # 16 column-tiles (of 128) per chunk
f32 = mybir.dt.float32

_cache = {}

# ablation switches (env-settable for HW debugging)
import os as _os
PACK_MULTI = bool(int(_os.environ.get("FCF_PACK_MULTI", "0")))
BCAST_PE = bool(int(_os.environ.get("FCF_BCAST_PE", "0")))
NO_RANK1 = bool(int(_os.environ.get("FCF_NO_RANK1", "0")))
# staged build-up: 0=memcpy 1=+t2rep*y 2=+s+c 3=+transpose/pack 4=+rank1
# 5=+loss (full kernel)
STAGE = int(_os.environ.get("FCF_STAGE", "5"))


def _build():
    nc = bacc.Bacc("TRN2", target_bir_lowering=False, debug=False,
                   num_devices=NCORES)
    Yd = nc.dram_tensor("y", (K, M), f32, kind="ExternalInput").ap()
    x2 = nc.dram_tensor("x2", (128, 2), f32, kind="ExternalInput").ap()
    xr = nc.dram_tensor("xr", (1, K), f32, kind="ExternalInput").ap()
    cnt = nc.dram_tensor("cnt", (128, 128), f32, kind="ExternalInput").ap()
    sl = nc.dram_tensor("sl", (128, 128), f32, kind="ExternalInput").ap()
    sl2 = nc.dram_tensor("sl2", (128, 128), f32, kind="ExternalInput").ap()
    wl2 = nc.dram_tensor("wl2", (128, 128), f32, kind="ExternalInput").ap()
    tch2 = nc.dram_tensor("tch2", (128, 128), f32, kind="ExternalInput").ap()
    t2r = nc.dram_tensor("t2r", (1, M), f32, kind="ExternalInput").ap()
    yg = nc.dram_tensor("yg", (K, M), f32, kind="ExternalOutput").ap()
    pl = nc.dram_tensor("pl", (1, 1), f32, kind="ExternalOutput").ap()

    mult = mybir.AluOpType.mult
    add = mybir.AluOpType.add
    sub = mybir.AluOpType.subtract

    with tile.TileContext(nc) as tc:
        with (
            tc.tile_pool(name="ybuf", bufs=3) as ypool,
            tc.tile_pool(name="t2b", bufs=2) as t2pool,
            tc.tile_pool(name="ctb", bufs=2) as ctpool,
            tc.tile_pool(name="small", bufs=1) as small,
            tc.tile_pool(name="ps", bufs=1, space="PSUM") as pspool,
            tc.tile_pool(name="psx", bufs=1, space="PSUM") as xcpool,
        ):
            x2_sb = small.tile([128, 2], f32, tag="x2")
            nc.sync.dma_start(x2_sb[:], x2[:])
            xr_sb = small.tile([1, K], f32, tag="xr")
            nc.sync.dma_start(xr_sb[:], xr[:])
            cnt_sb = small.tile([128, 128], f32, tag="cnt")
            nc.sync.dma_start(cnt_sb[:], cnt[:])
            sl_sb = small.tile([128, 128], f32, tag="sl")
            nc.sync.dma_start(sl_sb[:], sl[:])
            sl2_sb = small.tile([128, 128], f32, tag="sl2")
            nc.sync.dma_start(sl2_sb[:], sl2[:])
            wl2_sb = small.tile([128, 128], f32, tag="wl2")
            nc.sync.dma_start(wl2_sb[:], wl2[:])
            tch2_sb = small.tile([128, 128], f32, tag="tch2")
            nc.sync.dma_start(tch2_sb[:], tch2[:])
            t2r_sb = small.tile([1, M], f32, tag="t2r")
            nc.sync.dma_start(t2r_sb[:], t2r[:])

            s_sb = small.tile([128, 128], f32, tag="s")
            c_sb = small.tile([128, 128], f32, tag="c")
            crow = small.tile([1, M], f32, tag="crow")
            ident = small.tile([128, 128], f32, tag="id")
            make_identity(nc, ident[:])
            ones = small.tile([128, 1], f32, tag="ones")
            nc.vector.memset(ones[:], 1.0)
            ones_row = small.tile([1, 128], f32, tag="onesr")
            nc.vector.memset(ones_row[:], 1.0)

            for q in range(NCH):
                msl = slice(q * F, (q + 1) * F)
                y0 = ypool.tile([128, F], f32, tag="y0")
                y1 = ypool.tile([128, F], f32, tag="y1")
                nc.sync.dma_start(y0[:], Yd[0:128, msl])
                nc.sync.dma_start(y1[:], Yd[128:256, msl])

                csl = slice(q * JT, (q + 1) * JT)
                if STAGE >= 2:
                    # s columns for this chunk: s[j-tile] = Y_tile.T @ x
                    spsum = pspool.tile([128, JT], f32, tag="sp")
                    for t in range(JT):
                        nc.tensor.matmul(spsum[:, t:t + 1],
                                         lhsT=y0[:, t * 128:(t + 1) * 128],
                                         rhs=x2_sb[:, 0:1], start=True,
                                         stop=False)
                        nc.tensor.matmul(spsum[:, t:t + 1],
                                         lhsT=y1[:, t * 128:(t + 1) * 128],
                                         rhs=x2_sb[:, 1:2], start=False,
                                         stop=True)
                    nc.scalar.copy(s_sb[:, csl], spsum[:])

                    # c = s*tch2 - wl2 (column-tiled layout)
                    nc.vector.tensor_tensor(out=c_sb[:, csl],
                                            in0=s_sb[:, csl],
                                            in1=tch2_sb[:, csl], op=mult)
                    nc.vector.tensor_tensor(out=c_sb[:, csl],
                                            in0=c_sb[:, csl],
                                            in1=wl2_sb[:, csl], op=sub)

                if STAGE >= 3:
                    # c tiled [128, JT] -> row fragment crow[0, q*F:(q+1)*F]
                    ctp = pspool.tile([JT, 128], f32, tag="ctp")
                    nc.tensor.transpose(out=ctp[:], in_=c_sb[:, csl],
                                        identity=ident[:])
                    ctsb = ctpool.tile([JT, 128], f32, tag="ctsb")
                    nc.scalar.copy(ctsb[:], ctp[:])
                    if PACK_MULTI:
                        for j in range(JT):
                            nc.sync.dma_start(
                                crow[0:1,
                                     q * F + j * 128:q * F + (j + 1) * 128],
                                ctsb[j:j + 1, :])
                    else:
                        nc.sync.dma_start(crow[0:1, msl], ctsb[:])

                # replicate t2 row chunk across 128 partitions (DRAM src:
                # SBUF sources reject zero partition stride)
                t2rep = t2pool.tile([128, F], f32, tag="t2rep")
                if STAGE < 1:
                    pass
                elif BCAST_PE:
                    for b in range(F // 512):
                        bsl = slice(b * 512, (b + 1) * 512)
                        rsl = slice(q * F + b * 512, q * F + (b + 1) * 512)
                        t2pp = xcpool.tile([128, 512], f32, tag="t2pp")
                        nc.tensor.matmul(t2pp[:], lhsT=ones_row[:],
                                         rhs=t2r_sb[0:1, rsl],
                                         start=True, stop=True)
                        nc.scalar.copy(t2rep[:, bsl], t2pp[:])
                else:
                    nc.sync.dma_start(t2rep[:],
                                      t2r[0:1, msl].to_broadcast([128, F]))

                # pass A: y *= t2rep (in place)
                if STAGE >= 1:
                    nc.vector.tensor_tensor(out=y0[:], in0=y0[:],
                                            in1=t2rep[:], op=mult)
                    nc.vector.tensor_tensor(out=y1[:], in0=y1[:],
                                            in1=t2rep[:], op=mult)

                # rank-1: xc = X_half (x) c_chunk via PE, then y += xc
                xc = xcpool.tile([128, F], f32, tag="xc")
                rank1_on = STAGE >= 4 and not NO_RANK1
                for h, ytile in (((0, y0), (1, y1)) if rank1_on else ()):
                    for b in range(F // 512):
                        bsl = slice(b * 512, (b + 1) * 512)
                        rsl = slice(q * F + b * 512, q * F + (b + 1) * 512)
                        nc.tensor.matmul(xc[:, bsl],
                                         lhsT=xr_sb[0:1, h * 128:(h + 1) * 128],
                                         rhs=crow[0:1, rsl],
                                         start=True, stop=True)
                    nc.vector.tensor_tensor(out=ytile[:], in0=ytile[:],
                                            in1=xc[:], op=add)

                nc.sync.dma_start(yg[0:128, msl], y0[:])
                nc.sync.dma_start(yg[128:256, msl], y1[:])

            # loss partial: sum(cnt*s^2 - 2*sl*s + sl2) over this shard
            if STAGE < 5:
                plsb = small.tile([1, 1], f32, tag="plsb")
                nc.vector.memset(plsb[:], 0.0)
                nc.sync.dma_start(pl[:], plsb[:])
            else:
                lt = small.tile([128, 128], f32, tag="lt")
                a1 = small.tile([128, 1], f32, tag="a1")
                a2 = small.tile([128, 1], f32, tag="a2")
                a3 = small.tile([128, 1], f32, tag="a3")
                nc.vector.tensor_tensor(out=lt[:], in0=cnt_sb[:], in1=s_sb[:],
                                        op=mult)
                nc.vector.tensor_tensor(out=lt[:], in0=lt[:], in1=s_sb[:],
                                        op=mult)
                nc.vector.tensor_reduce(out=a1[:], in_=lt[:],
                                        axis=mybir.AxisListType.X, op=add)
                nc.vector.tensor_tensor(out=lt[:], in0=sl_sb[:], in1=s_sb[:],
                                        op=mult)
                nc.vector.tensor_reduce(out=a2[:], in_=lt[:],
                                        axis=mybir.AxisListType.X, op=add)
                nc.vector.tensor_reduce(out=a3[:], in_=sl2_sb[:],
                                        axis=mybir.AxisListType.X,
                                        op=add)
                nc.vector.tensor_scalar_mul(a2[:], a2[:], -2.0)
                nc.vector.tensor_add(out=a1[:], in0=a1[:], in1=a2[:])
                nc.vector.tensor_add(out=a1[:], in0=a1[:], in1=a3[:])
                plp = pspool.tile([1, 1], f32, tag="plp")
                nc.tensor.matmul(plp[:], lhsT=a1[:], rhs=ones[:], start=True,
                                 stop=True)
                plsb = small.tile([1, 1], f32, tag="plsb")
                nc.scalar.copy(plsb[:], plp[:])
                nc.sync.dma_start(pl[:], plsb[:])

    nc.compile()
    return nc


def _prep_inputs(X, global_Y, likes, movie_ids):
    X = np.asarray(X, np.float32)
    Y = np.asarray(global_Y, np.float32)
    likes = np.asarray(likes, np.float32)
    ids = np.asarray(movie_ids).astype(np.int64)

    cnt = np.bincount(ids, minlength=I).astype(np.float32)
    likes64 = likes.astype(np.float64)
    sl = np.bincount(ids, weights=likes64, minlength=I).astype(np.float32)
    sl2 = np.bincount(ids, weights=likes64 * likes64,
                      minlength=I).astype(np.float32)
    touched = cnt > 0
    lidx = np.zeros(I, np.int64)
    lidx[ids] = np.arange(N)          # last occurrence wins
    win_like = np.where(touched, likes[lidx], 0.0).astype(np.float32)
    tch2 = (touched * (2.0 / N)).astype(np.float32)
    wl2 = (win_like.astype(np.float64) * tch2).astype(np.float32)
    t2 = (touched * (2.0 * LAMBDA_REG / N)).astype(np.float32)

    x2 = np.ascontiguousarray(X.reshape(2, 128).T)
    xr = np.ascontiguousarray(X.reshape(1, K))

    def pack(a):
        # local column m = j*128 + p  ->  [p, j]
        return np.ascontiguousarray(a.reshape(128, 128).T)

    in_maps = []
    for d in range(NCORES):
        sl_d = slice(d * M, (d + 1) * M)
        in_maps.append({
            "y": np.ascontiguousarray(Y[:, sl_d]),
            "x2": x2,
            "xr": xr,
            "cnt": pack(cnt[sl_d]),
            "sl": pack(sl[sl_d]),
            "sl2": pack(sl2[sl_d]),
            "wl2": pack(wl2[sl_d]),
            "tch2": pack(tch2[sl_d]),
            "t2r": np.ascontiguousarray(t2[sl_d].reshape(1, M)),
        })
    return in_maps


def kernel_run(X, global_Y, likes, movie_ids, trace=False, tmpdir=None):
    if "nc" not in _cache:
        _cache["nc"] = _build()
    nc = _cache["nc"]
    in_maps = _prep_inputs(X, global_Y, likes, movie_ids)
    res = bass_utils.run_bass_kernel_spmd(
        nc, in_maps, core_ids=list(range(NCORES)), trace=trace, tmpdir=tmpdir)
    y_grad = np.concatenate([res.results[d]["yg"] for d in range(NCORES)],
                            axis=1)
    loss = np.float32(
        sum(float(res.results[d]["pl"][0, 0]) for d in range(NCORES)) / N)
    return (loss, y_grad), res


def kernel(X, global_Y, likes, movie_ids):
    out, _ = kernel_run(X, global_Y, likes, movie_ids, trace=False)
    return out


# revision 19
# speedup vs baseline: 1.4164x; 1.4164x over previous
"""Trainium2 Bass kernel for nn_FCFClient (scatter_memory).

reference(X, global_Y, likes, movie_ids) -> (avg_loss, y_grad):
    s     = X @ global_Y                    # (I,) per-item scores
    err_j = s[id_j] - likes_j               # per interaction
    loss  = mean(err^2)
    y_grad[:, m] = (2*err_win(m)*X.T + 2*lam*Y[:, m]) / N   for touched m (last
                   occurrence wins, CPU-jax scatter semantics), else 0.

All per-interaction work is compressed host-side (pure index manipulation /
bincount over ids+likes) into dense per-column arrays so that
    loss = (sum_m cnt*s^2 - 2*sl*s + sl2) / N
    y_grad[:, m] = c[m]*X.T + t2[m]*Y[:, m]
        c[m]  = s[m]*tch2[m] - wl2[m],  tch2 = touched*2/N, wl2 = win_like*tch2
        t2[m] = touched[m]*2*lam/N

Device (per core, 16384-column shard): stream Y; PE computes s with X as the
stationary operand (1-column weight loads; fp32 matmul is two-pass LOW/HIGH on
this PE so fat weight loads are poison); s bounces through DRAM into a
column-tiled layout for the loss reduction and for c; c returns to row layout
via one PE transpose + pack-DMA; GPSIMD partition_broadcast replicates t2 and
c across partitions; DVE applies y = y*t2rep then y = crep*X_col + y (fused
scalar_tensor_tensor). Memory roofline: read Y + write y_grad.
"""

import numpy as np

import concourse.bass as bass
import concourse.tile as tile
from concourse import bacc, mybir, bass_utils
from concourse.masks import make_identity

K = 256
I = 131072
N = 262144
NCORES = 8
M = I // NCORES          # 16384 columns per core
LAMBDA_REG = 0.01
F = 2048                 # columns per chunk
NCH = M // F             # 8 chunks
JT = F // 128            # 16 column-tiles (of 128) per chunk
f32 = mybir.dt.float32

_cache = {}


def _build():
    nc = bacc.Bacc("TRN2", target_bir_lowering=False, debug=False,
                   num_devices=NCORES)
    Yd = nc.dram_tensor("y", (K, M), f32, kind="ExternalInput").ap()
    x2 = nc.dram_tensor("x2", (128, 2), f32, kind="ExternalInput").ap()
    cnt = nc.dram_tensor("cnt", (128, 128), f32, kind="ExternalInput").ap()
    sl = nc.dram_tensor("sl", (128, 128), f32, kind="ExternalInput").ap()
    sl2 = nc.dram_tensor("sl2", (128, 128), f32, kind="ExternalInput").ap()
    wl2 = nc.dram_tensor("wl2", (128, 128), f32, kind="ExternalInput").ap()
    tch2 = nc.dram_tensor("tch2", (128, 128), f32, kind="ExternalInput").ap()
    t2r = nc.dram_tensor("t2r", (1, M), f32, kind="ExternalInput").ap()
    yg = nc.dram_tensor("yg", (K, M), f32, kind="ExternalOutput").ap()
    pl = nc.dram_tensor("pl", (1, 1), f32, kind="ExternalOutput").ap()

    mult = mybir.AluOpType.mult
    add = mybir.AluOpType.add
    sub = mybir.AluOpType.subtract

    with tile.TileContext(nc) as tc:
        with (
            tc.tile_pool(name="ybuf", bufs=3) as ypool,
            tc.tile_pool(name="crepb", bufs=2) as crepool,
            tc.tile_pool(name="ctb", bufs=2) as ctpool,
            tc.tile_pool(name="small", bufs=1) as small,
            tc.tile_pool(name="ps", bufs=2, space="PSUM") as pspool,
            tc.tile_pool(name="dram", bufs=1, space="DRAM") as drampool,
        ):
            x2_sb = small.tile([128, 2], f32, tag="x2")
            nc.sync.dma_start(x2_sb[:], x2[:])
            cnt_sb = small.tile([128, 128], f32, tag="cnt")
            nc.sync.dma_start(cnt_sb[:], cnt[:])
            sl_sb = small.tile([128, 128], f32, tag="sl")
            nc.sync.dma_start(sl_sb[:], sl[:])
            sl2_sb = small.tile([128, 128], f32, tag="sl2")
            nc.sync.dma_start(sl2_sb[:], sl2[:])
            wl2_sb = small.tile([128, 128], f32, tag="wl2")
            nc.sync.dma_start(wl2_sb[:], wl2[:])
            tch2_sb = small.tile([128, 128], f32, tag="tch2")
            nc.sync.dma_start(tch2_sb[:], tch2[:])
            s_sb = small.tile([128, 128], f32, tag="s")
            c_sb = small.tile([128, 128], f32, tag="c")
            ident = small.tile([128, 128], f32, tag="id")
            make_identity(nc, ident[:])
            ones = small.tile([128, 1], f32, tag="ones")
            nc.vector.memset(ones[:], 1.0)
            s_dram = drampool.tile([1, M], f32, tag="sd")

            for q in range(NCH):
                msl = slice(q * F, (q + 1) * F)
                y0 = ypool.tile([128, F], f32, tag="y0")
                y1 = ypool.tile([128, F], f32, tag="y1")
                nc.sync.dma_start(y0[:], Yd[0:128, msl])
                nc.sync.dma_start(y1[:], Yd[128:256, msl])

                # s row chunk: X stationary (1-col weights), Y moving
                srow = ctpool.tile([1, F], f32, tag="srow")
                for b in range(F // 512):
                    bsl = slice(b * 512, (b + 1) * 512)
                    sp = pspool.tile([1, 512], f32, tag="sp")
                    nc.tensor.matmul(sp[:], lhsT=x2_sb[:, 0:1],
                                     rhs=y0[:, bsl], start=True, stop=False)
                    nc.tensor.matmul(sp[:], lhsT=x2_sb[:, 1:2],
                                     rhs=y1[:, bsl], start=False, stop=True)
                    nc.scalar.copy(srow[0:1, bsl], sp[:])

                # bounce s chunk through DRAM into column-tiled layout
                nc.sync.dma_start(s_dram[0:1, msl], srow[:])
                csl = slice(q * JT, (q + 1) * JT)
                nc.sync.dma_start(
                    s_sb[:, csl],
                    s_dram[0:1, msl].rearrange("o (j p) -> (o p) j", p=128))

                # c = s*tch2 - wl2 (column-tiled)
                nc.vector.tensor_tensor(out=c_sb[:, csl], in0=s_sb[:, csl],
                                        in1=tch2_sb[:, csl], op=mult)
                nc.vector.tensor_tensor(out=c_sb[:, csl], in0=c_sb[:, csl],
                                        in1=wl2_sb[:, csl], op=sub)

                # c tiled -> row fragment
                ctp = pspool.tile([JT, 128], f32, tag="ctp")
                nc.tensor.transpose(out=ctp[:], in_=c_sb[:, csl],
                                    identity=ident[:])
                ctsb = ctpool.tile([JT, 128], f32, tag="ctsb")
                nc.scalar.copy(ctsb[:], ctp[:])
                crow = ctpool.tile([1, F], f32, tag="crow")
                nc.sync.dma_start(crow[:], ctsb[:])

                # replicate c and t2 chunks across partitions (gpsimd)
                crep = crepool.tile([128, F], f32, tag="crep")
                nc.gpsimd.partition_broadcast(crep[:], crow[:])
                t2row = ctpool.tile([1, F], f32, tag="t2row")
                nc.sync.dma_start(t2row[:], t2r[0:1, msl])
                t2rep = crepool.tile([128, F], f32, tag="t2rep")
                nc.gpsimd.partition_broadcast(t2rep[:], t2row[:])

                # y = y*t2rep ; y = crep*X_col + y
                for h, ytile in ((0, y0), (1, y1)):
                    nc.vector.tensor_tensor(out=ytile[:], in0=ytile[:],
                                            in1=t2rep[:], op=mult)
                    nc.vector.scalar_tensor_tensor(
                        out=ytile[:], in0=crep[:],
                        scalar=x2_sb[:, h:h + 1], in1=ytile[:],
                        op0=mult, op1=add)

                nc.sync.dma_start(yg[0:128, msl], y0[:])
                nc.sync.dma_start(yg[128:256, msl], y1[:])

            # loss partial: sum(cnt*s^2 - 2*sl*s + sl2) over this shard
            lt = small.tile([128, 128], f32, tag="lt")
            a1 = small.tile([128, 1], f32, tag="a1")
            a2 = small.tile([128, 1], f32, tag="a2")
            a3 = small.tile([128, 1], f32, tag="a3")
            nc.vector.tensor_tensor(out=lt[:], in0=cnt_sb[:], in1=s_sb[:],
                                    op=mult)
            nc.vector.tensor_tensor(out=lt[:], in0=lt[:], in1=s_sb[:],
                                    op=mult)
            nc.vector.tensor_reduce(out=a1[:], in_=lt[:],
                                    axis=mybir.AxisListType.X, op=add)
            nc.vector.tensor_tensor(out=lt[:], in0=sl_sb[:], in1=s_sb[:],
                                    op=mult)
            nc.vector.tensor_reduce(out=a2[:], in_=lt[:],
                                    axis=mybir.AxisListType.X, op=add)
            nc.vector.tensor_reduce(out=a3[:], in_=sl2_sb[:],
                                    axis=mybir.AxisListType.X, op=add)
            nc.vector.tensor_scalar_mul(a2[:], a2[:], -2.0)
            nc.vector.tensor_add(out=a1[:], in0=a1[:], in1=a2[:])
            nc.vector.tensor_add(out=a1[:], in0=a1[:], in1=a3[:])
            plp = pspool.tile([1, 1], f32, tag="sp")
            nc.tensor.matmul(plp[:], lhsT=a1[:], rhs=ones[:], start=True,
                             stop=True)
            plsb = small.tile([1, 1], f32, tag="plsb")
            nc.scalar.copy(plsb[:], plp[:])
            nc.sync.dma_start(pl[:], plsb[:])

    nc.compile()
    return nc


def _prep_inputs(X, global_Y, likes, movie_ids):
    X = np.asarray(X, np.float32)
    Y = np.asarray(global_Y, np.float32)
    likes = np.asarray(likes, np.float32)
    ids = np.asarray(movie_ids).astype(np.int64)

    cnt = np.bincount(ids, minlength=I).astype(np.float32)
    likes64 = likes.astype(np.float64)
    sl = np.bincount(ids, weights=likes64, minlength=I).astype(np.float32)
    sl2 = np.bincount(ids, weights=likes64 * likes64,
                      minlength=I).astype(np.float32)
    touched = cnt > 0
    lidx = np.zeros(I, np.int64)
    lidx[ids] = np.arange(N)          # last occurrence wins
    win_like = np.where(touched, likes[lidx], 0.0).astype(np.float32)
    tch2 = (touched * (2.0 / N)).astype(np.float32)
    wl2 = (win_like.astype(np.float64) * tch2).astype(np.float32)
    t2 = (touched * (2.0 * LAMBDA_REG / N)).astype(np.float32)

    x2 = np.ascontiguousarray(X.reshape(2, 128).T)

    def pack(a):
        # local column m = j*128 + p  ->  [p, j]
        return np.ascontiguousarray(a.reshape(128, 128).T)

    in_maps = []
    for d in range(NCORES):
        sl_d = slice(d * M, (d + 1) * M)
        in_maps.append({
            "y": np.ascontiguousarray(Y[:, sl_d]),
            "x2": x2,
            "cnt": pack(cnt[sl_d]),
            "sl": pack(sl[sl_d]),
            "sl2": pack(sl2[sl_d]),
            "wl2": pack(wl2[sl_d]),
            "tch2": pack(tch2[sl_d]),
            "t2r": np.ascontiguousarray(t2[sl_d].reshape(1, M)),
        })
    return in_maps


def kernel_run(X, global_Y, likes, movie_ids, trace=False, tmpdir=None):
    if "nc" not in _cache:
        _cache["nc"] = _build()
    nc = _cache["nc"]
    in_maps = _prep_inputs(X, global_Y, likes, movie_ids)
    res = bass_utils.run_bass_kernel_spmd(
        nc, in_maps, core_ids=list(range(NCORES)), trace=trace, tmpdir=tmpdir)
    y_grad = np.concatenate([res.results[d]["yg"] for d in range(NCORES)],
                            axis=1)
    loss = np.float32(
        sum(float(res.results[d]["pl"][0, 0]) for d in range(NCORES)) / N)
    return (loss, y_grad), res


def kernel(X, global_Y, likes, movie_ids):
    out, _ = kernel_run(X, global_Y, likes, movie_ids, trace=False)
    return out


# revision 23
# speedup vs baseline: 1.5119x; 1.0674x over previous
"""Trainium2 Bass kernel for nn_FCFClient (scatter_memory).

reference(X, global_Y, likes, movie_ids) -> (avg_loss, y_grad):
    s     = X @ global_Y                    # (I,) per-item scores
    err_j = s[id_j] - likes_j               # per interaction
    loss  = mean(err^2)
    y_grad[:, m] = (2*err_win(m)*X.T + 2*lam*Y[:, m]) / N   for touched m (last
                   occurrence wins, CPU-jax scatter semantics), else 0.

All per-interaction work is compressed host-side (pure index manipulation /
bincount over ids+likes) into dense per-column arrays so that
    loss = (sum_m cnt*s^2 - 2*sl*s + sl2) / N
    y_grad[:, m] = c[m]*X.T + t2[m]*Y[:, m]
        c[m]  = s[m]*tch2[m] - wl2[m],  tch2 = touched*2/N, wl2 = win_like*tch2
        t2[m] = touched[m]*2*lam/N

Device (per core, 16384-column shard): stream Y; PE computes s with X as the
stationary operand (1-column weight loads; fp32 matmul is two-pass LOW/HIGH on
this PE so fat weight loads are poison); s bounces through DRAM into a
column-tiled layout for the loss reduction and for c; c returns to row layout
via one PE transpose + pack-DMA; GPSIMD partition_broadcast replicates t2 and
c across partitions; DVE applies y = y*t2rep then y = crep*X_col + y (fused
scalar_tensor_tensor). Memory roofline: read Y + write y_grad.
"""

import numpy as np

import concourse.bass as bass
import concourse.tile as tile
from concourse import bacc, mybir, bass_utils
from concourse.masks import make_identity

K = 256
I = 131072
N = 262144
NCORES = 8
M = I // NCORES          # 16384 columns per core
LAMBDA_REG = 0.01
F = 2048                 # columns per chunk
NCH = M // F             # 8 chunks
JT = F // 128            # 16 column-tiles (of 128) per chunk
f32 = mybir.dt.float32

_cache = {}


def _build():
    nc = bacc.Bacc("TRN2", target_bir_lowering=False, debug=False,
                   num_devices=NCORES)
    Yd = nc.dram_tensor("y", (K, M), f32, kind="ExternalInput").ap()
    x2 = nc.dram_tensor("x2", (128, 2), f32, kind="ExternalInput").ap()
    cnt = nc.dram_tensor("cnt", (128, 128), f32, kind="ExternalInput").ap()
    sl = nc.dram_tensor("sl", (128, 128), f32, kind="ExternalInput").ap()
    sl2 = nc.dram_tensor("sl2", (128, 128), f32, kind="ExternalInput").ap()
    wl2 = nc.dram_tensor("wl2", (128, 128), f32, kind="ExternalInput").ap()
    tch2 = nc.dram_tensor("tch2", (128, 128), f32, kind="ExternalInput").ap()
    t2r = nc.dram_tensor("t2r", (1, M), f32, kind="ExternalInput").ap()
    yg = nc.dram_tensor("yg", (K, M), f32, kind="ExternalOutput").ap()
    pl = nc.dram_tensor("pl", (1, 1), f32, kind="ExternalOutput").ap()

    mult = mybir.AluOpType.mult
    add = mybir.AluOpType.add
    sub = mybir.AluOpType.subtract

    with tile.TileContext(nc) as tc:
        with (
            tc.tile_pool(name="ybuf", bufs=4) as ypool,
            tc.tile_pool(name="crepb", bufs=3) as crepool,
            tc.tile_pool(name="ctb", bufs=3) as ctpool,
            tc.tile_pool(name="small", bufs=1) as small,
            tc.tile_pool(name="ps", bufs=4, space="PSUM") as pspool,
            tc.tile_pool(name="dram", bufs=1, space="DRAM") as drampool,
        ):
            x2_sb = small.tile([128, 2], f32, tag="x2")
            nc.sync.dma_start(x2_sb[:], x2[:])
            cnt_sb = small.tile([128, 128], f32, tag="cnt")
            nc.sync.dma_start(cnt_sb[:], cnt[:])
            sl_sb = small.tile([128, 128], f32, tag="sl")
            nc.sync.dma_start(sl_sb[:], sl[:])
            sl2_sb = small.tile([128, 128], f32, tag="sl2")
            nc.sync.dma_start(sl2_sb[:], sl2[:])
            wl2_sb = small.tile([128, 128], f32, tag="wl2")
            nc.sync.dma_start(wl2_sb[:], wl2[:])
            tch2_sb = small.tile([128, 128], f32, tag="tch2")
            nc.sync.dma_start(tch2_sb[:], tch2[:])
            s_sb = small.tile([128, 128], f32, tag="s")
            ident = small.tile([128, 128], f32, tag="id")
            make_identity(nc, ident[:])
            ones = small.tile([128, 1], f32, tag="ones")
            nc.vector.memset(ones[:], 1.0)
            s_dram = drampool.tile([1, M], f32, tag="sd")

            for q in range(NCH):
                msl = slice(q * F, (q + 1) * F)
                y0 = ypool.tile([128, F], f32, tag="y0")
                y1 = ypool.tile([128, F], f32, tag="y1")
                nc.sync.dma_start(y0[:], Yd[0:128, msl])
                nc.sync.dma_start(y1[:], Yd[128:256, msl])

                # s row chunk: X stationary (1-col weights), Y moving
                srow = ctpool.tile([1, F], f32, tag="srow")
                for b in range(F // 512):
                    bsl = slice(b * 512, (b + 1) * 512)
                    sp = pspool.tile([1, 512], f32, tag="sp")
                    nc.tensor.matmul(sp[:], lhsT=x2_sb[:, 0:1],
                                     rhs=y0[:, bsl], start=True, stop=False)
                    nc.tensor.matmul(sp[:], lhsT=x2_sb[:, 1:2],
                                     rhs=y1[:, bsl], start=False, stop=True)
                    nc.scalar.copy(srow[0:1, bsl], sp[:])

                # bounce s chunk through DRAM into column-tiled layout
                nc.sync.dma_start(s_dram[0:1, msl], srow[:])
                csl = slice(q * JT, (q + 1) * JT)
                stl = ctpool.tile([128, JT], f32, tag="stl")
                nc.sync.dma_start(
                    stl[:],
                    s_dram[0:1, msl].rearrange("o (j p) -> (o p) j", p=128))
                # persistent copy for the end-of-kernel loss reduction
                nc.scalar.copy(s_sb[:, csl], stl[:])

                # c = s*tch2 - wl2 (column-tiled)
                ctl = ctpool.tile([128, JT], f32, tag="ctl")
                nc.vector.tensor_tensor(out=ctl[:], in0=stl[:],
                                        in1=tch2_sb[:, csl], op=mult)
                nc.vector.tensor_tensor(out=ctl[:], in0=ctl[:],
                                        in1=wl2_sb[:, csl], op=sub)

                # c tiled -> row fragment
                ctp = pspool.tile([JT, 128], f32, tag="ctp")
                nc.tensor.transpose(out=ctp[:], in_=ctl[:],
                                    identity=ident[:])
                ctsb = ctpool.tile([JT, 128], f32, tag="ctsb")
                nc.scalar.copy(ctsb[:], ctp[:])
                crow = ctpool.tile([1, F], f32, tag="crow")
                nc.sync.dma_start(crow[:], ctsb[:])

                # replicate c and t2 chunks across partitions (gpsimd)
                crep = crepool.tile([128, F], f32, tag="crep")
                nc.gpsimd.partition_broadcast(crep[:], crow[:])
                t2row = ctpool.tile([1, F], f32, tag="t2row")
                nc.sync.dma_start(t2row[:], t2r[0:1, msl])
                t2rep = crepool.tile([128, F], f32, tag="t2rep")
                nc.gpsimd.partition_broadcast(t2rep[:], t2row[:])

                # y = y*t2rep ; y = crep*X_col + y
                for h, ytile in ((0, y0), (1, y1)):
                    nc.vector.tensor_tensor(out=ytile[:], in0=ytile[:],
                                            in1=t2rep[:], op=mult)
                    nc.vector.scalar_tensor_tensor(
                        out=ytile[:], in0=crep[:],
                        scalar=x2_sb[:, h:h + 1], in1=ytile[:],
                        op0=mult, op1=add)

                nc.sync.dma_start(yg[0:128, msl], y0[:])
                nc.sync.dma_start(yg[128:256, msl], y1[:])

            # loss partial: sum(cnt*s^2 - 2*sl*s + sl2) over this shard
            lt = small.tile([128, 128], f32, tag="lt")
            a1 = small.tile([128, 1], f32, tag="a1")
            a2 = small.tile([128, 1], f32, tag="a2")
            a3 = small.tile([128, 1], f32, tag="a3")
            nc.vector.tensor_tensor(out=lt[:], in0=cnt_sb[:], in1=s_sb[:],
                                    op=mult)
            nc.vector.tensor_tensor(out=lt[:], in0=lt[:], in1=s_sb[:],
                                    op=mult)
            nc.vector.tensor_reduce(out=a1[:], in_=lt[:],
                                    axis=mybir.AxisListType.X, op=add)
            nc.vector.tensor_tensor(out=lt[:], in0=sl_sb[:], in1=s_sb[:],
                                    op=mult)
            nc.vector.tensor_reduce(out=a2[:], in_=lt[:],
                                    axis=mybir.AxisListType.X, op=add)
            nc.vector.tensor_reduce(out=a3[:], in_=sl2_sb[:],
                                    axis=mybir.AxisListType.X, op=add)
            nc.vector.tensor_scalar_mul(a2[:], a2[:], -2.0)
            nc.vector.tensor_add(out=a1[:], in0=a1[:], in1=a2[:])
            nc.vector.tensor_add(out=a1[:], in0=a1[:], in1=a3[:])
            plp = pspool.tile([1, 1], f32, tag="sp")
            nc.tensor.matmul(plp[:], lhsT=a1[:], rhs=ones[:], start=True,
                             stop=True)
            plsb = small.tile([1, 1], f32, tag="plsb")
            nc.scalar.copy(plsb[:], plp[:])
            nc.sync.dma_start(pl[:], plsb[:])

    nc.compile()
    return nc


def _prep_inputs(X, global_Y, likes, movie_ids):
    X = np.asarray(X, np.float32)
    Y = np.asarray(global_Y, np.float32)
    likes = np.asarray(likes, np.float32)
    ids = np.asarray(movie_ids).astype(np.int64)

    cnt = np.bincount(ids, minlength=I).astype(np.float32)
    likes64 = likes.astype(np.float64)
    sl = np.bincount(ids, weights=likes64, minlength=I).astype(np.float32)
    sl2 = np.bincount(ids, weights=likes64 * likes64,
                      minlength=I).astype(np.float32)
    touched = cnt > 0
    lidx = np.zeros(I, np.int64)
    lidx[ids] = np.arange(N)          # last occurrence wins
    win_like = np.where(touched, likes[lidx], 0.0).astype(np.float32)
    tch2 = (touched * (2.0 / N)).astype(np.float32)
    wl2 = (win_like.astype(np.float64) * tch2).astype(np.float32)
    t2 = (touched * (2.0 * LAMBDA_REG / N)).astype(np.float32)

    x2 = np.ascontiguousarray(X.reshape(2, 128).T)

    def pack(a):
        # local column m = j*128 + p  ->  [p, j]
        return np.ascontiguousarray(a.reshape(128, 128).T)

    in_maps = []
    for d in range(NCORES):
        sl_d = slice(d * M, (d + 1) * M)
        in_maps.append({
            "y": np.ascontiguousarray(Y[:, sl_d]),
            "x2": x2,
            "cnt": pack(cnt[sl_d]),
            "sl": pack(sl[sl_d]),
            "sl2": pack(sl2[sl_d]),
            "wl2": pack(wl2[sl_d]),
            "tch2": pack(tch2[sl_d]),
            "t2r": np.ascontiguousarray(t2[sl_d].reshape(1, M)),
        })
    return in_maps


def kernel_run(X, global_Y, likes, movie_ids, trace=False, tmpdir=None):
    if "nc" not in _cache:
        _cache["nc"] = _build()
    nc = _cache["nc"]
    in_maps = _prep_inputs(X, global_Y, likes, movie_ids)
    res = bass_utils.run_bass_kernel_spmd(
        nc, in_maps, core_ids=list(range(NCORES)), trace=trace, tmpdir=tmpdir)
    y_grad = np.concatenate([res.results[d]["yg"] for d in range(NCORES)],
                            axis=1)
    loss = np.float32(
        sum(float(res.results[d]["pl"][0, 0]) for d in range(NCORES)) / N)
    return (loss, y_grad), res


def kernel(X, global_Y, likes, movie_ids):
    out, _ = kernel_run(X, global_Y, likes, movie_ids, trace=False)
    return out
